# revision 18
# baseline (speedup 1.0000x reference)
"""Deformable-attention Trainium2 Bass kernel.

Contract: kernel(**inputs) takes FULL inputs (np arrays, shapes per spec) and
returns the FULL output [8,128,96,96] f32. Internally: data-parallel over the
batch dim across 8 NeuronCores (one batch element per core), SPMD program via
bass_utils.run_bass_kernel_spmd.

Per-core algorithm (validated against the jax reference in numpy first):
  1. attention logits = 3x3 conv(query) via 9 shifted matmuls over a padded
     query plane (PE, fp32r), + bias; softmax over the 8 points is folded into
     the sample weights (exp on ACT, sum/recip on DVE).
  2. value is transposed to pixel-major and written to a padded DRAM table
     vpad[h]: slot 1 + (y+1)*96 + x = value[h,:,y,x]; rows y=-1,96 and a lead
     slot are zeros, so out-of-range rows gather zeros.
  3. per (head,point) sample: one dma_gather of the top pixel-pair (elem =
     2 pixels x 64ch = 512B, elem_step = 1 pixel) and one of the bottom pair
     (same int16 index tensor, table offset +96 slots).  x-edge wraps gather
     in-plane garbage which is zeroed by validity-masked weights.
  4. weighted accumulate: 4 chained scalar_tensor_tensor ops per 128-pixel
     chunk per point (per-partition scalars = attn*bilinear*valid weights).
  5. 1x1 proj: PE transpose of acc chunks + fp32r matmul + bias, DMA out.
"""

import os
import sys
import dataclasses

import numpy as np

for _p in ("/opt/trn_rl_repo",):
    if _p not in sys.path and os.path.isdir(_p):
        sys.path.insert(0, _p)

C = 128
H = W = 96
HW = H * W          # 9216
NH, NP, HD = 2, 8, 64
NCH = 72            # 128-pixel chunks per plane
PW = 98             # padded conv plane side
NPIX_PAD = PW * PW  # 9604
QPADN = 99 + NPIX_PAD + 99  # 9802
TBL = 9411          # vpad table rows
TBLSZ = (TBL + 2) * 64  # tail guard so [[64,TBL],[1,128]] view fits

_NC_CACHE = {}


def _rep_matrix_np():
    # R_all[:, p*128+m] = 1 iff k == p*16 + (m % 16); matmul out[m,:] = in[p*16+m%16,:]
    R = np.zeros((128, 8 * 128), np.float32)
    for p in range(8):
        for m in range(128):
            R[p * 16 + (m % 16), p * 128 + m] = 1.0
    return R


def build_nc(loop_k: int = 1, skip_stt: bool = False, skip_gather: bool = False):
    from concourse import bass, mybir, bacc, tile

    f32 = mybir.dt.float32
    f32r = mybir.dt.float32r
    i16 = mybir.dt.int16
    Alu = mybir.AluOpType
    Act = mybir.ActivationFunctionType

    nc = bacc.Bacc(None, target_bir_lowering=False)

    query = nc.dram_tensor("query", [C, H, W], f32, kind="ExternalInput")
    value = nc.dram_tensor("value", [C, HW], f32, kind="ExternalInput")
    rp = nc.dram_tensor("rp", [HW, 32], f32, kind="ExternalInput")
    attn_w = nc.dram_tensor("attn_w", [16, C, 9], f32, kind="ExternalInput")
    attn_b = nc.dram_tensor("attn_b", [16, 1], f32, kind="ExternalInput")
    proj_w = nc.dram_tensor("proj_w", [C, C], f32, kind="ExternalInput")
    proj_b = nc.dram_tensor("proj_b", [C, 1], f32, kind="ExternalInput")
    gidx = nc.dram_tensor("gidx", [16, 128, 576], i16, kind="ExternalInput")
    gcoord = nc.dram_tensor("gcoord", [HW, 32], f32, kind="ExternalInput")
    out = nc.dram_tensor("out", [C, HW], f32, kind="ExternalOutput")

    vpad = nc.dram_tensor("vpad", [NH, TBLSZ], f32)  # internal scratch

    eye_d = nc.inline_tensor(np.eye(128, dtype=np.float32), name="eye128")
    rall_d = nc.inline_tensor(_rep_matrix_np(), name="repmat")

    import contextlib

    with tile.TileContext(nc) as tc:
        with (
            tc.tile_pool(name="const", bufs=1) as pc,
            tc.tile_pool(name="persist", bufs=1) as pp,
            (tc.For_i(0, loop_k, 1) if loop_k > 1 else contextlib.nullcontext()),
        ):
            # ---- constants to SBUF ----
            eye = pc.tile([128, 128], f32)
            nc.sync.dma_start(eye[:], eye_d[:, :])
            rall = pc.tile([128, 8 * 128], f32)
            nc.sync.dma_start(rall[:], rall_d[:, :])
            wconv = pc.tile([128, 16, 9], f32)
            nc.sync.dma_start(wconv[:], attn_w[:, :, :].rearrange("o i t -> i o t"))
            pnat = pc.tile([128, 128], f32)
            nc.sync.dma_start(pnat[:], proj_w[:, :])
            projw = pc.tile([128, 128], f32)
            attnb = pc.tile([16, 1], f32)
            nc.sync.dma_start(attnb[:], attn_b[:, :])
            projb = pc.tile([128, 1], f32)
            nc.sync.dma_start(projb[:], proj_b[:, :])
            zero64 = pc.tile([128, 64], f32)
            nc.vector.memset(zero64[:], 0.0)

            # ---- persistent ----
            idxrep = pp.tile([128, 16, 576], i16)   # (h*8+p) -> wrapped idx slots
            nc.sync.dma_start(
                idxrep[:], gidx[:, :, :].rearrange("a p s -> p a s"))
            wq = pp.tile([128, NH, 4, NCH, NP], f32)  # quarter weights
            acc = pp.tile([128, NCH, 128], f32)       # weighted sums, pix-major
            if skip_stt:
                nc.vector.memset(acc[:], 0.0)
            apix = pp.tile([128, NCH, 16], f32)       # exp(logits), pix-major
            recr = pp.tile([128, NH, NCH, NP], f32)   # 1/sum(exp) replicated over p

            # =============== phase A: vpad table build ===============
            with (
                tc.tile_pool(name="phA", bufs=1) as pa,
                tc.tile_pool(name="psA", bufs=4, space="PSUM") as psA,
            ):
                v_sb = pa.tile([128, NCH, 128], f32)
                nc.sync.dma_start(v_sb[:], value[:, :].rearrange("p (c n) -> p c n", n=128))
                ppt = psA.tile([128, 128], f32, tag="ppw")
                nc.tensor.transpose(ppt[:], pnat[:], eye[:])
                nc.scalar.copy(projw[:], ppt[:])
                vt = pa.tile([128, NCH, 128], f32)
                for c in range(NCH):
                    pt = psA.tile([128, 128], f32, tag="pvt")
                    nc.tensor.transpose(pt[:], v_sb[:, c, :], eye[:])
                    nc.scalar.copy(vt[:, c, :], pt[:])
                # zeros for pad slots
                for h in range(NH):
                    base = vpad[h, :]
                    dst_main = dataclasses.replace(
                        base, ap=[[64, 128], [8192, NCH], [1, 64]],
                        offset=base.offset + 97 * 64)
                    nc.sync.dma_start(dst_main, vt[:, :, h * 64:(h + 1) * 64])
                    dst_z0 = dataclasses.replace(
                        base, ap=[[64, 97], [1, 64]])
                    nc.sync.dma_start(dst_z0, zero64[0:97, :])
                    dst_z1 = dataclasses.replace(
                        base, ap=[[64, 100], [1, 64]], offset=base.offset + 9313 * 64)
                    nc.sync.dma_start(dst_z1, zero64[0:100, :])

            # =============== phase B: conv + attn transpose/exp ===============
            with (
                tc.tile_pool(name="phB", bufs=1) as pb,
                tc.tile_pool(name="psB", bufs=2, space="PSUM") as psB,
            ):
                qpad = pb.tile([128, QPADN], f32)
                nc.vector.memset(qpad[:], 0.0)
                # interior: row y -> elements [198 + y*98, +96)
                dst_int = qpad[:, 198:198 + 96 * 98].rearrange(
                    "p (a b) -> p a b", b=98)[:, :, 0:96]
                nc.sync.dma_start(dst_int, query[:, :, :])

                attn_sb = pb.tile([16, H, W], f32)
                chunks = [(r0, min(5, 98 - r0)) for r0 in range(0, 98, 5)]
                for (r0, nrows) in chunks:
                    ncols = nrows * PW
                    pcv = psB.tile([16, 5, PW], f32, tag="pconv")
                    pcv_flat = pcv[:].rearrange("p a b -> p (a b)")
                    base = 99 + r0 * PW
                    for t in range(9):
                        dy, dx = t // 3 - 1, t % 3 - 1
                        sh = dy * PW + dx
                        nc.tensor.matmul(
                            pcv_flat[:, 0:ncols],
                            wconv[:, :, t],
                            qpad[:, base + sh: base + sh + ncols],
                            start=(t == 0), stop=(t == 8),
                        )
                    rr0, rr1 = max(r0, 1), min(r0 + nrows, 97)
                    if rr1 > rr0:
                        nc.scalar.activation(
                            attn_sb[:, rr0 - 1: rr1 - 1, :],
                            pcv[:, rr0 - r0: rr1 - r0, 1:97],
                            Act.Identity, bias=attnb[:, 0:1], scale=1.0)
                attn_flat = attn_sb[:].rearrange("p a b -> p (a b)")
                for c in range(NCH):
                    pat = psB.tile([128, 16], f32, tag="pattn")
                    nc.tensor.transpose(
                        pat[:], attn_flat[:, c * 128:(c + 1) * 128], eye[0:16, 0:16])
                    nc.scalar.activation(apix[:, c, :], pat[:], Act.Exp)

            # softmax denominators (on exp'd, pixel-major attn)
            with tc.tile_pool(name="phSM", bufs=1) as psm:
                sums = psm.tile([128, NCH, NH], f32)
                rec = psm.tile([128, NCH, NH], f32)
                for h in range(NH):
                    nc.vector.tensor_reduce(
                        sums[:, :, h: h + 1], apix[:, :, h * 8:(h + 1) * 8],
                        mybir.AxisListType.X, Alu.add)
                    nc.vector.reciprocal(rec[:, :, h: h + 1], sums[:, :, h: h + 1])
                    for p in range(NP):
                        nc.vector.tensor_copy(
                            recr[:, h, :, p: p + 1], rec[:, :, h: h + 1])

            # =============== phase C: coords -> weights ===============
            with tc.tile_pool(name="phC", bufs=1) as pcc:
                rpn = pcc.tile([128, NCH, 32], f32)
                rp_src = dataclasses.replace(
                    rp[:, :].rearrange("a b -> (a b)"),
                    ap=[[32, 128], [4096, NCH], [1, 32]])
                nc.sync.dma_start(rpn[:], rp_src)
                rpn_r = rpn[:].rearrange("p c (h k x) -> p c h k x", h=2, k=8, x=2)
                gco = pcc.tile([128, NCH, 32], f32)
                gco_src = dataclasses.replace(
                    gcoord[:, :].rearrange("a b -> (a b)"),
                    ap=[[32, 128], [4096, NCH], [1, 32]])
                nc.sync.dma_start(gco[:], gco_src)
                gco_r = gco[:].rearrange("p c (h k x) -> p c h k x", h=2, k=8, x=2)
                for h in range(NH):
                    cx = rpn_r[:, :, h, :, 0]
                    cy = rpn_r[:, :, h, :, 1]
                    xs = pcc.tile([128, NCH, NP], f32, tag="xs")
                    ys = pcc.tile([128, NCH, NP], f32, tag="ys")
                    nc.vector.tensor_scalar(xs[:], cx, float(W), 0.5, Alu.mult, Alu.add)
                    nc.vector.tensor_scalar(ys[:], cy, float(H), 0.5, Alu.mult, Alu.add)
                    gx = gco_r[:, :, h, :, 0]
                    gy = gco_r[:, :, h, :, 1]
                    wx = pcc.tile([128, NCH, NP], f32, tag="wx")
                    wy = pcc.tile([128, NCH, NP], f32, tag="wy")
                    nc.vector.tensor_tensor(wx[:], xs[:], gx, Alu.subtract)
                    nc.vector.tensor_tensor(wy[:], ys[:], gy, Alu.subtract)
                    vl = pcc.tile([128, NCH, NP], f32, tag="vl")
                    vr = pcc.tile([128, NCH, NP], f32, tag="vr")
                    nc.vector.tensor_scalar(vl[:], gx, 1.0, None, Alu.is_ge)
                    nc.vector.tensor_scalar(vr[:], gx, 95.0, None, Alu.is_le)
                    omwx = pcc.tile([128, NCH, NP], f32, tag="omwx")
                    omwy = pcc.tile([128, NCH, NP], f32, tag="omwy")
                    nc.vector.tensor_scalar(omwx[:], wx[:], -1.0, 1.0, Alu.mult, Alu.add)
                    nc.vector.tensor_scalar(omwy[:], wy[:], -1.0, 1.0, Alu.mult, Alu.add)
                    xlw = pcc.tile([128, NCH, NP], f32, tag="xlw")
                    xrw = pcc.tile([128, NCH, NP], f32, tag="xrw")
                    nc.vector.tensor_tensor(xlw[:], omwx[:], vl[:], Alu.mult)
                    nc.vector.tensor_tensor(xrw[:], wx[:], vr[:], Alu.mult)
                    an = pcc.tile([128, NCH, NP], f32, tag="an")
                    nc.vector.tensor_tensor(
                        an[:], apix[:, :, h * 8:(h + 1) * 8], recr[:, h, :, :], Alu.mult)
                    ty = pcc.tile([128, NCH, NP], f32, tag="ty")
                    by = pcc.tile([128, NCH, NP], f32, tag="by")
                    nc.vector.tensor_tensor(ty[:], an[:], omwy[:], Alu.mult)
                    nc.vector.tensor_tensor(by[:], an[:], wy[:], Alu.mult)
                    nc.vector.tensor_tensor(wq[:, h, 0, :, :], ty[:], xlw[:], Alu.mult)
                    nc.vector.tensor_tensor(wq[:, h, 1, :, :], ty[:], xrw[:], Alu.mult)
                    nc.vector.tensor_tensor(wq[:, h, 2, :, :], by[:], xlw[:], Alu.mult)
                    nc.vector.tensor_tensor(wq[:, h, 3, :, :], by[:], xrw[:], Alu.mult)

            # =============== phase D: gathers + weighted accumulate + proj ===============
            with (
                tc.tile_pool(name="phD", bufs=2) as pd,
                tc.tile_pool(name="psD", bufs=2, space="PSUM") as psD,
            ):
                for h in range(NH):
                    gtop = dataclasses.replace(
                        vpad[h, :], ap=[[64, TBL], [1, 128]])
                    gbot = dataclasses.replace(
                        vpad[h, :], ap=[[64, TBL - 96], [1, 128]],
                        offset=vpad[h, :].offset + 96 * 64)
                    for p in range(NP):
                        for half in range(2):
                            T = pd.tile([128, 36, 128], f32, tag="T")
                            Bt = pd.tile([128, 36, 128], f32, tag="B")
                            idxs = idxrep[:, h * 8 + p, half * 288:(half + 1) * 288]
                            if not skip_gather:
                                nc.gpsimd.dma_gather(
                                    T[:], gtop, idxs, 4608, 4608,
                                    elem_size=128, elem_step=64,
                                    single_packet=False)
                                nc.gpsimd.dma_gather(
                                    Bt[:], gbot, idxs, 4608, 4608,
                                    elem_size=128, elem_step=64,
                                    single_packet=False)
                            else:
                                nc.vector.memset(T[:, 0, 0:2], 0.0)
                                nc.vector.memset(Bt[:, 0, 0:2], 0.0)
                            if skip_stt:
                                continue
                            for cl in range(36):
                                c = half * 36 + cl
                                dsts = acc[:, c, h * 64:(h + 1) * 64]
                                first = zero64[:, :] if p == 0 else dsts
                                nc.vector.scalar_tensor_tensor(
                                    dsts, T[:, cl, 0:64], wq[:, h, 0, c, p: p + 1],
                                    first, Alu.mult, Alu.add)
                                nc.vector.scalar_tensor_tensor(
                                    dsts, T[:, cl, 64:128], wq[:, h, 1, c, p: p + 1],
                                    dsts, Alu.mult, Alu.add)
                                nc.vector.scalar_tensor_tensor(
                                    dsts, Bt[:, cl, 0:64], wq[:, h, 2, c, p: p + 1],
                                    dsts, Alu.mult, Alu.add)
                                nc.vector.scalar_tensor_tensor(
                                    dsts, Bt[:, cl, 64:128], wq[:, h, 3, c, p: p + 1],
                                    dsts, Alu.mult, Alu.add)

                # ---- proj ----
                for g4 in range(18):
                    wt4 = pd.tile([128, 512], f32, tag="wt4")
                    for j in range(4):
                        c = g4 * 4 + j
                        ptt = psD.tile([128, 128], f32, tag="ptrans")
                        nc.tensor.transpose(ptt[:], acc[:, c, :], eye[:])
                        nc.scalar.copy(wt4[:, j * 128:(j + 1) * 128], ptt[:])
                    po = psD.tile([128, 512], f32, tag="pproj")
                    nc.tensor.matmul(
                        po[:], projw[:], wt4[:])
                    osb = pd.tile([128, 512], f32, tag="osb")
                    nc.scalar.activation(
                        osb[:], po[:], Act.Identity, bias=projb[:, 0:1], scale=1.0)
                    nc.sync.dma_start(out[:, g4 * 512:(g4 + 1) * 512], osb[:])

    nc.compile()
    return nc


def _get_nc():
    if "nc" not in _NC_CACHE:
        _NC_CACHE["nc"] = build_nc()
    return _NC_CACHE["nc"]


def _make_in_maps(inputs):
    q = np.ascontiguousarray(np.asarray(inputs["query"], dtype=np.float32))
    v = np.ascontiguousarray(np.asarray(inputs["value"], dtype=np.float32))
    rp = np.ascontiguousarray(np.asarray(inputs["reference_points"], dtype=np.float32))
    aw = np.ascontiguousarray(
        np.asarray(inputs["attn_w"], dtype=np.float32).reshape(16, C, 9))
    ab = np.asarray(inputs["attn_b"], dtype=np.float32).reshape(16, 1)
    pw = np.ascontiguousarray(
        np.asarray(inputs["proj_w"], dtype=np.float32).reshape(C, C))
    pb = np.asarray(inputs["proj_b"], dtype=np.float32).reshape(C, 1)

    in_maps = []
    for b in range(8):
        rpb = rp[b].reshape(HW, NH, NP, 2)
        x0p1 = np.floor(rpb[..., 0] * W + 0.5)
        y0p1 = np.floor(rpb[..., 1] * H + 0.5)
        idx = (y0p1 * 96 + x0p1).astype(np.int16)       # [HW, NH, NP]
        gc = np.empty((HW, NH, NP, 2), np.float32)
        gc[..., 0] = x0p1
        gc[..., 1] = y0p1
        gc = np.ascontiguousarray(gc.reshape(HW, 32))
        # wrapped+replicated gather idx tensor: G[h*8+p, r, c*8+g] =
        #   idx[pix = c*128 + g*16 + (r%16), h, p]
        it = idx.reshape(NCH, 8, 16, NH, NP)             # [c, g, q, h, p]
        G = np.transpose(it, (3, 4, 2, 0, 1)).reshape(NH * NP, 1, 16, NCH * 8)
        G = np.broadcast_to(G, (NH * NP, 8, 16, NCH * 8))
        G = np.ascontiguousarray(
            G.reshape(NH * NP, 128, NCH * 8)).astype(np.int16)
        if os.environ.get("CONST_IDX") == "1":
            G = np.zeros_like(G)
        in_maps.append({
            "query": q[b],
            "value": v[b].reshape(C, HW),
            "rp": rp[b].reshape(HW, 32),
            "attn_w": aw,
            "attn_b": ab,
            "proj_w": pw,
            "proj_b": pb,
            "gidx": G,
            "gcoord": gc,
        })
    return in_maps


def kernel(**inputs):
    nc = _get_nc()
    from concourse.bass_utils import run_bass_kernel_spmd

    in_maps = _make_in_maps(inputs)
    res = run_bass_kernel_spmd(nc, in_maps, list(range(8)))
    _NC_CACHE["exec_time_ns"] = res.exec_time_ns
    _NC_CACHE["mean_exec_time_ns"] = res.mean_exec_time_ns
    _NC_CACHE["profile_json"] = res.profile_json
    outs = [res.results[b]["out"].reshape(C, H, W) for b in range(8)]
    return np.stack(outs).astype(np.float32)


if __name__ == "__main__":
    nc = build_nc()
    n = sum(len(bb.instructions) for bb in nc.main_func.blocks)
    print("built ok, instructions:", n)


# revision 19
# speedup vs baseline: 2.5347x; 2.5347x over previous
"""Deformable-attention Trainium2 Bass kernel.

Contract: kernel(**inputs) takes FULL inputs (np arrays, shapes per spec) and
returns the FULL output [8,128,96,96] f32. Internally: data-parallel over the
batch dim across 8 NeuronCores (one batch element per core), SPMD program via
bass_utils.run_bass_kernel_spmd.

Per-core algorithm (validated against the jax reference in numpy first):
  1. attention logits = 3x3 conv(query) via 9 shifted matmuls over a padded
     query plane (PE, fp32r), + bias; softmax over the 8 points is folded into
     the sample weights (exp on ACT, sum/recip on DVE).
  2. value is transposed to pixel-major and written to a padded DRAM table
     vpad[h]: slot 1 + (y+1)*96 + x = value[h,:,y,x]; rows y=-1,96 and a lead
     slot are zeros, so out-of-range rows gather zeros.
  3. per (head,point) sample: one dma_gather of the top pixel-pair (elem =
     2 pixels x 64ch = 512B, elem_step = 1 pixel) and one of the bottom pair
     (same int16 index tensor, table offset +96 slots).  x-edge wraps gather
     in-plane garbage which is zeroed by validity-masked weights.
  4. weighted accumulate: 4 chained scalar_tensor_tensor ops per 128-pixel
     chunk per point (per-partition scalars = attn*bilinear*valid weights).
  5. 1x1 proj: PE transpose of acc chunks + fp32r matmul + bias, DMA out.
"""

import os
import sys
import dataclasses

import numpy as np

for _p in ("/opt/trn_rl_repo",):
    if _p not in sys.path and os.path.isdir(_p):
        sys.path.insert(0, _p)

C = 128
H = W = 96
HW = H * W          # 9216
NH, NP, HD = 2, 8, 64
NCH = 72            # 128-pixel chunks per plane
PW = 98             # padded conv plane side
NPIX_PAD = PW * PW  # 9604
QPADN = 99 + NPIX_PAD + 99  # 9802
TBL = 9411          # vpad table rows
TBLSZ = (TBL + 2) * 64  # tail guard so [[64,TBL],[1,128]] view fits

_NC_CACHE = {}


def _rep_matrix_np():
    # R_all[:, p*128+m] = 1 iff k == p*16 + (m % 16); matmul out[m,:] = in[p*16+m%16,:]
    R = np.zeros((128, 8 * 128), np.float32)
    for p in range(8):
        for m in range(128):
            R[p * 16 + (m % 16), p * 128 + m] = 1.0
    return R


def build_nc(loop_k: int = 1, skip_stt: bool = False, skip_gather: bool = False):
    from concourse import bass, mybir, bacc, tile

    f32 = mybir.dt.float32
    f32r = mybir.dt.float32r
    i16 = mybir.dt.int16
    Alu = mybir.AluOpType
    Act = mybir.ActivationFunctionType

    nc = bacc.Bacc(None, target_bir_lowering=False)

    query = nc.dram_tensor("query", [C, H, W], f32, kind="ExternalInput")
    value = nc.dram_tensor("value", [C, HW], f32, kind="ExternalInput")
    rp = nc.dram_tensor("rp", [HW, 32], f32, kind="ExternalInput")
    attn_w = nc.dram_tensor("attn_w", [16, C, 9], f32, kind="ExternalInput")
    attn_b = nc.dram_tensor("attn_b", [16, 1], f32, kind="ExternalInput")
    proj_w = nc.dram_tensor("proj_w", [C, C], f32, kind="ExternalInput")
    proj_b = nc.dram_tensor("proj_b", [C, 1], f32, kind="ExternalInput")
    gidx = nc.dram_tensor("gidx", [16, 128, 576], i16, kind="ExternalInput")
    gcoord = nc.dram_tensor("gcoord", [HW, 32], f32, kind="ExternalInput")
    out = nc.dram_tensor("out", [C, HW], f32, kind="ExternalOutput")

    vpad = nc.dram_tensor("vpad", [NH, TBLSZ], f32)  # internal scratch

    eye_d = nc.inline_tensor(np.eye(128, dtype=np.float32), name="eye128")
    rall_d = nc.inline_tensor(_rep_matrix_np(), name="repmat")

    import contextlib

    with tile.TileContext(nc) as tc:
        with (
            tc.tile_pool(name="const", bufs=1) as pc,
            tc.tile_pool(name="persist", bufs=1) as pp,
            (tc.For_i(0, loop_k, 1) if loop_k > 1 else contextlib.nullcontext()),
        ):
            # ---- constants to SBUF ----
            eye = pc.tile([128, 128], f32)
            nc.sync.dma_start(eye[:], eye_d[:, :])
            rall = pc.tile([128, 8 * 128], f32)
            nc.sync.dma_start(rall[:], rall_d[:, :])
            wconv = pc.tile([128, 16, 9], f32)
            nc.sync.dma_start(wconv[:], attn_w[:, :, :].rearrange("o i t -> i o t"))
            pnat = pc.tile([128, 128], f32)
            nc.sync.dma_start(pnat[:], proj_w[:, :])
            projw = pc.tile([128, 128], f32)
            attnb = pc.tile([16, 1], f32)
            nc.sync.dma_start(attnb[:], attn_b[:, :])
            projb = pc.tile([128, 1], f32)
            nc.sync.dma_start(projb[:], proj_b[:, :])
            zero64 = pc.tile([128, 64], f32)
            nc.vector.memset(zero64[:], 0.0)

            # ---- persistent ----
            idxrep = pp.tile([128, 16, 576], i16)   # (h*8+p) -> wrapped idx slots
            nc.sync.dma_start(
                idxrep[:], gidx[:, :, :].rearrange("a p s -> p a s"))
            wq = pp.tile([128, NH, 4, NCH, NP], f32)  # quarter weights
            acc = pp.tile([128, NCH, 128], f32)       # weighted sums, pix-major
            if skip_stt:
                nc.vector.memset(acc[:], 0.0)
            apix = pp.tile([128, NCH, 16], f32)       # exp(logits), pix-major
            recr = pp.tile([128, NH, NCH, NP], f32)   # 1/sum(exp) replicated over p

            # =============== phase A: vpad table build ===============
            with (
                tc.tile_pool(name="phA", bufs=1) as pa,
                tc.tile_pool(name="psA", bufs=4, space="PSUM") as psA,
            ):
                v_sb = pa.tile([128, NCH, 128], f32)
                nc.sync.dma_start(v_sb[:], value[:, :].rearrange("p (c n) -> p c n", n=128))
                ppt = psA.tile([128, 128], f32, tag="ppw")
                nc.tensor.transpose(ppt[:], pnat[:], eye[:])
                nc.scalar.copy(projw[:], ppt[:])
                vt = pa.tile([128, NCH, 128], f32)
                for c in range(NCH):
                    pt = psA.tile([128, 128], f32, tag="pvt")
                    nc.tensor.transpose(pt[:], v_sb[:, c, :], eye[:])
                    nc.scalar.copy(vt[:, c, :], pt[:])
                # zeros for pad slots
                for h in range(NH):
                    base = vpad[h, :]
                    dst_main = dataclasses.replace(
                        base, ap=[[64, 128], [8192, NCH], [1, 64]],
                        offset=base.offset + 97 * 64)
                    nc.sync.dma_start(dst_main, vt[:, :, h * 64:(h + 1) * 64])
                    dst_z0 = dataclasses.replace(
                        base, ap=[[64, 97], [1, 64]])
                    nc.sync.dma_start(dst_z0, zero64[0:97, :])
                    dst_z1 = dataclasses.replace(
                        base, ap=[[64, 100], [1, 64]], offset=base.offset + 9313 * 64)
                    nc.sync.dma_start(dst_z1, zero64[0:100, :])

            # =============== phase B: conv + attn transpose/exp ===============
            with (
                tc.tile_pool(name="phB", bufs=1) as pb,
                tc.tile_pool(name="psB", bufs=2, space="PSUM") as psB,
            ):
                qpad = pb.tile([128, QPADN], f32)
                nc.vector.memset(qpad[:], 0.0)
                # interior: row y -> elements [198 + y*98, +96)
                dst_int = qpad[:, 198:198 + 96 * 98].rearrange(
                    "p (a b) -> p a b", b=98)[:, :, 0:96]
                nc.sync.dma_start(dst_int, query[:, :, :])

                attn_sb = pb.tile([16, H, W], f32)
                chunks = [(r0, min(5, 98 - r0)) for r0 in range(0, 98, 5)]
                for (r0, nrows) in chunks:
                    ncols = nrows * PW
                    pcv = psB.tile([16, 5, PW], f32, tag="pconv")
                    pcv_flat = pcv[:].rearrange("p a b -> p (a b)")
                    base = 99 + r0 * PW
                    for t in range(9):
                        dy, dx = t // 3 - 1, t % 3 - 1
                        sh = dy * PW + dx
                        nc.tensor.matmul(
                            pcv_flat[:, 0:ncols],
                            wconv[:, :, t],
                            qpad[:, base + sh: base + sh + ncols],
                            start=(t == 0), stop=(t == 8),
                        )
                    rr0, rr1 = max(r0, 1), min(r0 + nrows, 97)
                    if rr1 > rr0:
                        nc.scalar.activation(
                            attn_sb[:, rr0 - 1: rr1 - 1, :],
                            pcv[:, rr0 - r0: rr1 - r0, 1:97],
                            Act.Identity, bias=attnb[:, 0:1], scale=1.0)
                attn_flat = attn_sb[:].rearrange("p a b -> p (a b)")
                for c in range(NCH):
                    pat = psB.tile([128, 16], f32, tag="pattn")
                    nc.tensor.transpose(
                        pat[:], attn_flat[:, c * 128:(c + 1) * 128], eye[0:16, 0:16])
                    nc.scalar.activation(apix[:, c, :], pat[:], Act.Exp)

            # softmax denominators (on exp'd, pixel-major attn)
            with tc.tile_pool(name="phSM", bufs=1) as psm:
                sums = psm.tile([128, NCH, NH], f32)
                rec = psm.tile([128, NCH, NH], f32)
                for h in range(NH):
                    nc.vector.tensor_reduce(
                        sums[:, :, h: h + 1], apix[:, :, h * 8:(h + 1) * 8],
                        mybir.AxisListType.X, Alu.add)
                    nc.vector.reciprocal(rec[:, :, h: h + 1], sums[:, :, h: h + 1])
                    for p in range(NP):
                        nc.vector.tensor_copy(
                            recr[:, h, :, p: p + 1], rec[:, :, h: h + 1])

            # =============== phase C: coords -> weights ===============
            with tc.tile_pool(name="phC", bufs=1) as pcc:
                rpn = pcc.tile([128, NCH, 32], f32)
                rp_src = dataclasses.replace(
                    rp[:, :].rearrange("a b -> (a b)"),
                    ap=[[32, 128], [4096, NCH], [1, 32]])
                nc.sync.dma_start(rpn[:], rp_src)
                rpn_r = rpn[:].rearrange("p c (h k x) -> p c h k x", h=2, k=8, x=2)
                gco = pcc.tile([128, NCH, 32], f32)
                gco_src = dataclasses.replace(
                    gcoord[:, :].rearrange("a b -> (a b)"),
                    ap=[[32, 128], [4096, NCH], [1, 32]])
                nc.sync.dma_start(gco[:], gco_src)
                gco_r = gco[:].rearrange("p c (h k x) -> p c h k x", h=2, k=8, x=2)
                for h in range(NH):
                    cx = rpn_r[:, :, h, :, 0]
                    cy = rpn_r[:, :, h, :, 1]
                    xs = pcc.tile([128, NCH, NP], f32, tag="xs")
                    ys = pcc.tile([128, NCH, NP], f32, tag="ys")
                    nc.vector.tensor_scalar(xs[:], cx, float(W), 0.5, Alu.mult, Alu.add)
                    nc.vector.tensor_scalar(ys[:], cy, float(H), 0.5, Alu.mult, Alu.add)
                    gx = gco_r[:, :, h, :, 0]
                    gy = gco_r[:, :, h, :, 1]
                    wx = pcc.tile([128, NCH, NP], f32, tag="wx")
                    wy = pcc.tile([128, NCH, NP], f32, tag="wy")
                    nc.vector.tensor_tensor(wx[:], xs[:], gx, Alu.subtract)
                    nc.vector.tensor_tensor(wy[:], ys[:], gy, Alu.subtract)
                    vl = pcc.tile([128, NCH, NP], f32, tag="vl")
                    vr = pcc.tile([128, NCH, NP], f32, tag="vr")
                    nc.vector.tensor_scalar(vl[:], gx, 1.0, None, Alu.is_ge)
                    nc.vector.tensor_scalar(vr[:], gx, 95.0, None, Alu.is_le)
                    omwx = pcc.tile([128, NCH, NP], f32, tag="omwx")
                    omwy = pcc.tile([128, NCH, NP], f32, tag="omwy")
                    nc.vector.tensor_scalar(omwx[:], wx[:], -1.0, 1.0, Alu.mult, Alu.add)
                    nc.vector.tensor_scalar(omwy[:], wy[:], -1.0, 1.0, Alu.mult, Alu.add)
                    xlw = pcc.tile([128, NCH, NP], f32, tag="xlw")
                    xrw = pcc.tile([128, NCH, NP], f32, tag="xrw")
                    nc.vector.tensor_tensor(xlw[:], omwx[:], vl[:], Alu.mult)
                    nc.vector.tensor_tensor(xrw[:], wx[:], vr[:], Alu.mult)
                    an = pcc.tile([128, NCH, NP], f32, tag="an")
                    nc.vector.tensor_tensor(
                        an[:], apix[:, :, h * 8:(h + 1) * 8], recr[:, h, :, :], Alu.mult)
                    ty = pcc.tile([128, NCH, NP], f32, tag="ty")
                    by = pcc.tile([128, NCH, NP], f32, tag="by")
                    nc.vector.tensor_tensor(ty[:], an[:], omwy[:], Alu.mult)
                    nc.vector.tensor_tensor(by[:], an[:], wy[:], Alu.mult)
                    nc.vector.tensor_tensor(wq[:, h, 0, :, :], ty[:], xlw[:], Alu.mult)
                    nc.vector.tensor_tensor(wq[:, h, 1, :, :], ty[:], xrw[:], Alu.mult)
                    nc.vector.tensor_tensor(wq[:, h, 2, :, :], by[:], xlw[:], Alu.mult)
                    nc.vector.tensor_tensor(wq[:, h, 3, :, :], by[:], xrw[:], Alu.mult)

            # =============== phase D: gathers + weighted accumulate + proj ===============
            with (
                tc.tile_pool(name="phD", bufs=2) as pd,
                tc.tile_pool(name="psD", bufs=2, space="PSUM") as psD,
            ):
                for h in range(NH):
                    gtop = dataclasses.replace(
                        vpad[h, :], ap=[[64, TBL], [1, 128]])
                    gbot = dataclasses.replace(
                        vpad[h, :], ap=[[64, TBL - 96], [1, 128]],
                        offset=vpad[h, :].offset + 96 * 64)
                    for p in range(NP):
                        for half in range(2):
                            T = pd.tile([128, 36, 128], f32, tag="T")
                            Bt = pd.tile([128, 36, 128], f32, tag="B")
                            idxs = idxrep[:, h * 8 + p, half * 288:(half + 1) * 288]
                            if not skip_gather:
                                nc.gpsimd.dma_gather(
                                    T[:], gtop, idxs, 4608, 4608,
                                    elem_size=128, elem_step=64,
                                    single_packet=False)
                                nc.gpsimd.dma_gather(
                                    Bt[:], gbot, idxs, 4608, 4608,
                                    elem_size=128, elem_step=64,
                                    single_packet=False)
                            else:
                                nc.vector.memset(T[:, 0, 0:2], 0.0)
                                nc.vector.memset(Bt[:, 0, 0:2], 0.0)
                            if skip_stt:
                                continue
                            for cl in range(36):
                                c = half * 36 + cl
                                dsts = acc[:, c, h * 64:(h + 1) * 64]
                                first = zero64[:, :] if p == 0 else dsts
                                nc.vector.scalar_tensor_tensor(
                                    dsts, T[:, cl, 0:64], wq[:, h, 0, c, p: p + 1],
                                    first, Alu.mult, Alu.add)
                                nc.vector.scalar_tensor_tensor(
                                    dsts, T[:, cl, 64:128], wq[:, h, 1, c, p: p + 1],
                                    dsts, Alu.mult, Alu.add)
                                nc.vector.scalar_tensor_tensor(
                                    dsts, Bt[:, cl, 0:64], wq[:, h, 2, c, p: p + 1],
                                    dsts, Alu.mult, Alu.add)
                                nc.vector.scalar_tensor_tensor(
                                    dsts, Bt[:, cl, 64:128], wq[:, h, 3, c, p: p + 1],
                                    dsts, Alu.mult, Alu.add)

                # ---- proj ----
                for g4 in range(18):
                    wt4 = pd.tile([128, 512], f32, tag="wt4")
                    for j in range(4):
                        c = g4 * 4 + j
                        ptt = psD.tile([128, 128], f32, tag="ptrans")
                        nc.tensor.transpose(ptt[:], acc[:, c, :], eye[:])
                        nc.scalar.copy(wt4[:, j * 128:(j + 1) * 128], ptt[:])
                    po = psD.tile([128, 512], f32, tag="pproj")
                    nc.tensor.matmul(
                        po[:], projw[:], wt4[:])
                    osb = pd.tile([128, 512], f32, tag="osb")
                    nc.scalar.activation(
                        osb[:], po[:], Act.Identity, bias=projb[:, 0:1], scale=1.0)
                    nc.sync.dma_start(out[:, g4 * 512:(g4 + 1) * 512], osb[:])

    nc.compile()
    return nc


def _get_nc():
    if "nc" not in _NC_CACHE:
        _NC_CACHE["nc"] = build_nc()
    return _NC_CACHE["nc"]


def _make_in_maps(inputs):
    q = np.ascontiguousarray(np.asarray(inputs["query"], dtype=np.float32))
    v = np.ascontiguousarray(np.asarray(inputs["value"], dtype=np.float32))
    rp = np.ascontiguousarray(np.asarray(inputs["reference_points"], dtype=np.float32))
    aw = np.ascontiguousarray(
        np.asarray(inputs["attn_w"], dtype=np.float32).reshape(16, C, 9))
    ab = np.asarray(inputs["attn_b"], dtype=np.float32).reshape(16, 1)
    pw = np.ascontiguousarray(
        np.asarray(inputs["proj_w"], dtype=np.float32).reshape(C, C))
    pb = np.asarray(inputs["proj_b"], dtype=np.float32).reshape(C, 1)

    in_maps = []
    for b in range(8):
        rpb = rp[b].reshape(HW, NH, NP, 2)
        x0p1 = np.floor(rpb[..., 0] * W + 0.5)
        y0p1 = np.floor(rpb[..., 1] * H + 0.5)
        idx = (y0p1 * 96 + x0p1).astype(np.int16)       # [HW, NH, NP]
        gc = np.empty((HW, NH, NP, 2), np.float32)
        gc[..., 0] = x0p1
        gc[..., 1] = y0p1
        gc = np.ascontiguousarray(gc.reshape(HW, 32))
        # wrapped+replicated gather idx tensor: G[h*8+p, r, c*8+g] =
        #   idx[pix = c*128 + g*16 + (r%16), h, p]
        it = idx.reshape(NCH, 8, 16, NH, NP)             # [c, g, q, h, p]
        G = np.transpose(it, (3, 4, 2, 0, 1)).reshape(NH * NP, 1, 16, NCH * 8)
        G = np.broadcast_to(G, (NH * NP, 8, 16, NCH * 8))
        G = np.ascontiguousarray(
            G.reshape(NH * NP, 128, NCH * 8)).astype(np.int16)
        if os.environ.get("CONST_IDX") == "1":
            G = np.zeros_like(G)
        elif os.environ.get("CONST_IDX") == "2":
            # sequential: sample i gathers row i -> max HBM locality
            c = np.arange(NCH)[:, None, None]
            g = np.arange(8)[None, :, None]
            qq = np.arange(16)[None, None, :]
            seq = (c * 128 + g * 16 + qq).astype(np.int16)  # [c,g,q]
            it2 = np.broadcast_to(seq[:, :, :, None, None], (NCH, 8, 16, NH, NP))
            G2 = np.transpose(it2, (3, 4, 2, 0, 1)).reshape(NH * NP, 1, 16, NCH * 8)
            G2 = np.broadcast_to(G2, (NH * NP, 8, 16, NCH * 8))
            G = np.ascontiguousarray(G2.reshape(NH * NP, 128, NCH * 8)).astype(np.int16)
        in_maps.append({
            "query": q[b],
            "value": v[b].reshape(C, HW),
            "rp": rp[b].reshape(HW, 32),
            "attn_w": aw,
            "attn_b": ab,
            "proj_w": pw,
            "proj_b": pb,
            "gidx": G,
            "gcoord": gc,
        })
    return in_maps


def kernel(**inputs):
    nc = _get_nc()
    from concourse.bass_utils import run_bass_kernel_spmd

    in_maps = _make_in_maps(inputs)
    res = run_bass_kernel_spmd(nc, in_maps, list(range(8)))
    _NC_CACHE["exec_time_ns"] = res.exec_time_ns
    _NC_CACHE["mean_exec_time_ns"] = res.mean_exec_time_ns
    _NC_CACHE["profile_json"] = res.profile_json
    outs = [res.results[b]["out"].reshape(C, H, W) for b in range(8)]
    return np.stack(outs).astype(np.float32)


if __name__ == "__main__":
    nc = build_nc()
    n = sum(len(bb.instructions) for bb in nc.main_func.blocks)
    print("built ok, instructions:", n)


# revision 20
# speedup vs baseline: 4.0856x; 1.6119x over previous
"""Deformable-attention Trainium2 Bass kernel.

Contract: kernel(**inputs) takes FULL inputs (np arrays, shapes per spec) and
returns the FULL output [8,128,96,96] f32. Internally: data-parallel over the
batch dim across 8 NeuronCores (one batch element per core), SPMD program via
bass_utils.run_bass_kernel_spmd.

Per-core algorithm (validated against the jax reference in numpy first):
  1. attention logits = 3x3 conv(query) via 9 shifted matmuls over a padded
     query plane (PE, fp32r), + bias; softmax over the 8 points is folded into
     the sample weights (exp on ACT, sum/recip on DVE).
  2. value is transposed to pixel-major and written to a padded DRAM table
     vpad[h]: slot 1 + (y+1)*96 + x = value[h,:,y,x]; rows y=-1,96 and a lead
     slot are zeros, so out-of-range rows gather zeros.
  3. per (head,point) sample: one dma_gather of the top pixel-pair (elem =
     2 pixels x 64ch = 512B, elem_step = 1 pixel) and one of the bottom pair
     (same int16 index tensor, table offset +96 slots).  x-edge wraps gather
     in-plane garbage which is zeroed by validity-masked weights.
  4. weighted accumulate: 4 chained scalar_tensor_tensor ops per 128-pixel
     chunk per point (per-partition scalars = attn*bilinear*valid weights).
  5. 1x1 proj: PE transpose of acc chunks + fp32r matmul + bias, DMA out.
"""

import os
import sys
import dataclasses

import numpy as np

for _p in ("/opt/trn_rl_repo",):
    if _p not in sys.path and os.path.isdir(_p):
        sys.path.insert(0, _p)

C = 128
H = W = 96
HW = H * W          # 9216
NH, NP, HD = 2, 8, 64
NCH = 72            # 128-pixel chunks per plane
PW = 98             # padded conv plane side
NPIX_PAD = PW * PW  # 9604
QPADN = 99 + NPIX_PAD + 99  # 9802
TBL = 9314          # dup-table entries (idx = y0p1*96+x0p1 in [0,9312], +1 read)
TBLSZ = (TBL + 2) * 128  # entry = [row r-1 (64ch) | row r (64ch)]; +2 guard entries

_NC_CACHE = {}


def _rep_matrix_np():
    # R_all[:, p*128+m] = 1 iff k == p*16 + (m % 16); matmul out[m,:] = in[p*16+m%16,:]
    R = np.zeros((128, 8 * 128), np.float32)
    for p in range(8):
        for m in range(128):
            R[p * 16 + (m % 16), p * 128 + m] = 1.0
    return R


def build_nc(loop_k: int = 1, skip_stt: bool = False, skip_gather: bool = False):
    from concourse import bass, mybir, bacc, tile

    f32 = mybir.dt.float32
    f32r = mybir.dt.float32r
    i16 = mybir.dt.int16
    Alu = mybir.AluOpType
    Act = mybir.ActivationFunctionType

    nc = bacc.Bacc(None, target_bir_lowering=False)

    query = nc.dram_tensor("query", [C, H, W], f32, kind="ExternalInput")
    value = nc.dram_tensor("value", [C, HW], f32, kind="ExternalInput")
    rp = nc.dram_tensor("rp", [HW, 32], f32, kind="ExternalInput")
    attn_w = nc.dram_tensor("attn_w", [16, C, 9], f32, kind="ExternalInput")
    attn_b = nc.dram_tensor("attn_b", [16, 1], f32, kind="ExternalInput")
    proj_w = nc.dram_tensor("proj_w", [C, C], f32, kind="ExternalInput")
    proj_b = nc.dram_tensor("proj_b", [C, 1], f32, kind="ExternalInput")
    gidx = nc.dram_tensor("gidx", [16, 128, 576], i16, kind="ExternalInput")
    gcoord = nc.dram_tensor("gcoord", [HW, 32], f32, kind="ExternalInput")
    out = nc.dram_tensor("out", [C, HW], f32, kind="ExternalOutput")

    vpad = nc.dram_tensor("vpad", [NH, TBLSZ], f32)  # internal scratch

    eye_d = nc.inline_tensor(np.eye(128, dtype=np.float32), name="eye128")
    rall_d = nc.inline_tensor(_rep_matrix_np(), name="repmat")

    import contextlib

    with tile.TileContext(nc) as tc:
        with (
            tc.tile_pool(name="const", bufs=1) as pc,
            tc.tile_pool(name="persist", bufs=1) as pp,
            (tc.For_i(0, loop_k, 1) if loop_k > 1 else contextlib.nullcontext()),
        ):
            # ---- constants to SBUF ----
            eye = pc.tile([128, 128], f32)
            nc.sync.dma_start(eye[:], eye_d[:, :])
            rall = pc.tile([128, 8 * 128], f32)
            nc.sync.dma_start(rall[:], rall_d[:, :])
            wconv = pc.tile([128, 16, 9], f32)
            nc.sync.dma_start(wconv[:], attn_w[:, :, :].rearrange("o i t -> i o t"))
            pnat = pc.tile([128, 128], f32)
            nc.sync.dma_start(pnat[:], proj_w[:, :])
            projw = pc.tile([128, 128], f32)
            attnb = pc.tile([16, 1], f32)
            nc.sync.dma_start(attnb[:], attn_b[:, :])
            projb = pc.tile([128, 1], f32)
            nc.sync.dma_start(projb[:], proj_b[:, :])
            zero64 = pc.tile([128, 64], f32)
            nc.vector.memset(zero64[:], 0.0)

            # ---- persistent ----
            idxrep = pp.tile([128, 16, 576], i16)   # (h*8+p) -> wrapped idx slots
            nc.sync.dma_start(
                idxrep[:], gidx[:, :, :].rearrange("a p s -> p a s"))
            wq = pp.tile([128, NH, 4, NCH, NP], f32)  # quarter weights
            acc = pp.tile([128, NCH, 128], f32)       # weighted sums, pix-major
            if skip_stt:
                nc.vector.memset(acc[:], 0.0)
            apix = pp.tile([128, NCH, 16], f32)       # exp(logits), pix-major
            recr = pp.tile([128, NH, NCH, NP], f32)   # 1/sum(exp) replicated over p

            # =============== phase A: vpad table build ===============
            with (
                tc.tile_pool(name="phA", bufs=1) as pa,
                tc.tile_pool(name="psA", bufs=4, space="PSUM") as psA,
            ):
                v_sb = pa.tile([128, NCH, 128], f32)
                nc.sync.dma_start(v_sb[:], value[:, :].rearrange("p (c n) -> p c n", n=128))
                ppt = psA.tile([128, 128], f32, tag="ppw")
                nc.tensor.transpose(ppt[:], pnat[:], eye[:])
                nc.scalar.copy(projw[:], ppt[:])
                vt = pa.tile([128, NCH, 128], f32)
                for c in range(NCH):
                    pt = psA.tile([128, 128], f32, tag="pvt")
                    nc.tensor.transpose(pt[:], v_sb[:, c, :], eye[:])
                    nc.scalar.copy(vt[:, c, :], pt[:])
                # dup-table: entry e=(rp,x)=1+rp*96+x holds
                #   [value row rp-1 | value row rp] (zeros out of range)
                for h in range(NH):
                    base = vpad[h, :]
                    hs = vt[:, :, h * 64:(h + 1) * 64]
                    # subslot 1 of entries 1+pix  <- value row(pix)
                    dA = dataclasses.replace(
                        base, ap=[[128, 128], [16384, NCH], [1, 64]],
                        offset=base.offset + 128 + 64)
                    nc.sync.dma_start(dA, hs)
                    # subslot 0 of entries 1+96+pix <- value row(pix)
                    dB = dataclasses.replace(
                        base, ap=[[128, 128], [16384, NCH], [1, 64]],
                        offset=base.offset + 97 * 128)
                    nc.sync.dma_start(dB, hs)
                    # zeros: entry 0 fully + sub0 of entries [1,97)
                    z0 = dataclasses.replace(
                        base, ap=[[128, 97], [1, 64]])
                    nc.sync.dma_start(z0, zero64[0:97, :])
                    z0b = dataclasses.replace(base, ap=[[1, 64]],
                                              offset=base.offset + 64)
                    nc.sync.dma_start(z0b, zero64[0:1, :])
                    # zeros: sub1 of entries [9217, 9316) (rows >= 96)
                    z1 = dataclasses.replace(
                        base, ap=[[128, 99], [1, 64]],
                        offset=base.offset + 9217 * 128 + 64)
                    nc.sync.dma_start(z1, zero64[0:99, :])
                    # zeros: sub0 of entries [9313, 9316)
                    z2 = dataclasses.replace(
                        base, ap=[[128, 3], [1, 64]],
                        offset=base.offset + 9313 * 128)
                    nc.sync.dma_start(z2, zero64[0:3, :])

            # =============== phase B: conv + attn transpose/exp ===============
            with (
                tc.tile_pool(name="phB", bufs=1) as pb,
                tc.tile_pool(name="psB", bufs=2, space="PSUM") as psB,
            ):
                qpad = pb.tile([128, QPADN], f32)
                nc.vector.memset(qpad[:], 0.0)
                # interior: row y -> elements [198 + y*98, +96)
                dst_int = qpad[:, 198:198 + 96 * 98].rearrange(
                    "p (a b) -> p a b", b=98)[:, :, 0:96]
                nc.sync.dma_start(dst_int, query[:, :, :])

                attn_sb = pb.tile([16, H, W], f32)
                chunks = [(r0, min(5, 98 - r0)) for r0 in range(0, 98, 5)]
                for (r0, nrows) in chunks:
                    ncols = nrows * PW
                    pcv = psB.tile([16, 5, PW], f32, tag="pconv")
                    pcv_flat = pcv[:].rearrange("p a b -> p (a b)")
                    base = 99 + r0 * PW
                    for t in range(9):
                        dy, dx = t // 3 - 1, t % 3 - 1
                        sh = dy * PW + dx
                        nc.tensor.matmul(
                            pcv_flat[:, 0:ncols],
                            wconv[:, :, t],
                            qpad[:, base + sh: base + sh + ncols],
                            start=(t == 0), stop=(t == 8),
                        )
                    rr0, rr1 = max(r0, 1), min(r0 + nrows, 97)
                    if rr1 > rr0:
                        nc.scalar.activation(
                            attn_sb[:, rr0 - 1: rr1 - 1, :],
                            pcv[:, rr0 - r0: rr1 - r0, 1:97],
                            Act.Identity, bias=attnb[:, 0:1], scale=1.0)
                attn_flat = attn_sb[:].rearrange("p a b -> p (a b)")
                for c in range(NCH):
                    pat = psB.tile([128, 16], f32, tag="pattn")
                    nc.tensor.transpose(
                        pat[:], attn_flat[:, c * 128:(c + 1) * 128], eye[0:16, 0:16])
                    nc.scalar.activation(apix[:, c, :], pat[:], Act.Exp)

            # softmax denominators (on exp'd, pixel-major attn)
            with tc.tile_pool(name="phSM", bufs=1) as psm:
                sums = psm.tile([128, NCH, NH], f32)
                rec = psm.tile([128, NCH, NH], f32)
                for h in range(NH):
                    nc.vector.tensor_reduce(
                        sums[:, :, h: h + 1], apix[:, :, h * 8:(h + 1) * 8],
                        mybir.AxisListType.X, Alu.add)
                    nc.vector.reciprocal(rec[:, :, h: h + 1], sums[:, :, h: h + 1])
                    for p in range(NP):
                        nc.vector.tensor_copy(
                            recr[:, h, :, p: p + 1], rec[:, :, h: h + 1])

            # =============== phase C: coords -> weights ===============
            with tc.tile_pool(name="phC", bufs=1) as pcc:
                rpn = pcc.tile([128, NCH, 32], f32)
                rp_src = dataclasses.replace(
                    rp[:, :].rearrange("a b -> (a b)"),
                    ap=[[32, 128], [4096, NCH], [1, 32]])
                nc.sync.dma_start(rpn[:], rp_src)
                rpn_r = rpn[:].rearrange("p c (h k x) -> p c h k x", h=2, k=8, x=2)
                gco = pcc.tile([128, NCH, 32], f32)
                gco_src = dataclasses.replace(
                    gcoord[:, :].rearrange("a b -> (a b)"),
                    ap=[[32, 128], [4096, NCH], [1, 32]])
                nc.sync.dma_start(gco[:], gco_src)
                gco_r = gco[:].rearrange("p c (h k x) -> p c h k x", h=2, k=8, x=2)
                for h in range(NH):
                    cx = rpn_r[:, :, h, :, 0]
                    cy = rpn_r[:, :, h, :, 1]
                    xs = pcc.tile([128, NCH, NP], f32, tag="xs")
                    ys = pcc.tile([128, NCH, NP], f32, tag="ys")
                    nc.vector.tensor_scalar(xs[:], cx, float(W), 0.5, Alu.mult, Alu.add)
                    nc.vector.tensor_scalar(ys[:], cy, float(H), 0.5, Alu.mult, Alu.add)
                    gx = gco_r[:, :, h, :, 0]
                    gy = gco_r[:, :, h, :, 1]
                    wx = pcc.tile([128, NCH, NP], f32, tag="wx")
                    wy = pcc.tile([128, NCH, NP], f32, tag="wy")
                    nc.vector.tensor_tensor(wx[:], xs[:], gx, Alu.subtract)
                    nc.vector.tensor_tensor(wy[:], ys[:], gy, Alu.subtract)
                    vl = pcc.tile([128, NCH, NP], f32, tag="vl")
                    vr = pcc.tile([128, NCH, NP], f32, tag="vr")
                    nc.vector.tensor_scalar(vl[:], gx, 1.0, None, Alu.is_ge)
                    nc.vector.tensor_scalar(vr[:], gx, 95.0, None, Alu.is_le)
                    omwx = pcc.tile([128, NCH, NP], f32, tag="omwx")
                    omwy = pcc.tile([128, NCH, NP], f32, tag="omwy")
                    nc.vector.tensor_scalar(omwx[:], wx[:], -1.0, 1.0, Alu.mult, Alu.add)
                    nc.vector.tensor_scalar(omwy[:], wy[:], -1.0, 1.0, Alu.mult, Alu.add)
                    xlw = pcc.tile([128, NCH, NP], f32, tag="xlw")
                    xrw = pcc.tile([128, NCH, NP], f32, tag="xrw")
                    nc.vector.tensor_tensor(xlw[:], omwx[:], vl[:], Alu.mult)
                    nc.vector.tensor_tensor(xrw[:], wx[:], vr[:], Alu.mult)
                    an = pcc.tile([128, NCH, NP], f32, tag="an")
                    nc.vector.tensor_tensor(
                        an[:], apix[:, :, h * 8:(h + 1) * 8], recr[:, h, :, :], Alu.mult)
                    ty = pcc.tile([128, NCH, NP], f32, tag="ty")
                    by = pcc.tile([128, NCH, NP], f32, tag="by")
                    nc.vector.tensor_tensor(ty[:], an[:], omwy[:], Alu.mult)
                    nc.vector.tensor_tensor(by[:], an[:], wy[:], Alu.mult)
                    nc.vector.tensor_tensor(wq[:, h, 0, :, :], ty[:], xlw[:], Alu.mult)
                    nc.vector.tensor_tensor(wq[:, h, 1, :, :], ty[:], xrw[:], Alu.mult)
                    nc.vector.tensor_tensor(wq[:, h, 2, :, :], by[:], xlw[:], Alu.mult)
                    nc.vector.tensor_tensor(wq[:, h, 3, :, :], by[:], xrw[:], Alu.mult)

            # =============== phase D: gathers + weighted accumulate + proj ===============
            with (
                tc.tile_pool(name="phD", bufs=2) as pd,
                tc.tile_pool(name="psD", bufs=2, space="PSUM") as psD,
            ):
                for h in range(NH):
                    gsrc = dataclasses.replace(
                        vpad[h, :], ap=[[128, TBL], [1, 256]])
                    for p in range(NP):
                        for half in range(2):
                            T = pd.tile([128, 36, 256], f32, tag="T")
                            idxs = idxrep[:, h * 8 + p, half * 288:(half + 1) * 288]
                            if not skip_gather:
                                nc.gpsimd.dma_gather(
                                    T[:], gsrc, idxs, 4608, 4608,
                                    elem_size=256, elem_step=128,
                                    single_packet=False)
                            else:
                                nc.vector.memset(T[:, 0, 0:2], 0.0)
                            if skip_stt:
                                continue
                            for cl in range(36):
                                c = half * 36 + cl
                                dsts = acc[:, c, h * 64:(h + 1) * 64]
                                first = zero64[:, :] if p == 0 else dsts
                                # patch free layout: [Ltop Lbot Rtop Rbot]
                                nc.vector.scalar_tensor_tensor(
                                    dsts, T[:, cl, 0:64], wq[:, h, 0, c, p: p + 1],
                                    first, Alu.mult, Alu.add)
                                nc.vector.scalar_tensor_tensor(
                                    dsts, T[:, cl, 128:192], wq[:, h, 1, c, p: p + 1],
                                    dsts, Alu.mult, Alu.add)
                                nc.vector.scalar_tensor_tensor(
                                    dsts, T[:, cl, 64:128], wq[:, h, 2, c, p: p + 1],
                                    dsts, Alu.mult, Alu.add)
                                nc.vector.scalar_tensor_tensor(
                                    dsts, T[:, cl, 192:256], wq[:, h, 3, c, p: p + 1],
                                    dsts, Alu.mult, Alu.add)

                # ---- proj ----
                for g4 in range(18):
                    wt4 = pd.tile([128, 512], f32, tag="wt4")
                    for j in range(4):
                        c = g4 * 4 + j
                        ptt = psD.tile([128, 128], f32, tag="ptrans")
                        nc.tensor.transpose(ptt[:], acc[:, c, :], eye[:])
                        nc.scalar.copy(wt4[:, j * 128:(j + 1) * 128], ptt[:])
                    po = psD.tile([128, 512], f32, tag="pproj")
                    nc.tensor.matmul(
                        po[:], projw[:], wt4[:])
                    osb = pd.tile([128, 512], f32, tag="osb")
                    nc.scalar.activation(
                        osb[:], po[:], Act.Identity, bias=projb[:, 0:1], scale=1.0)
                    nc.sync.dma_start(out[:, g4 * 512:(g4 + 1) * 512], osb[:])

    nc.compile()
    return nc


def _get_nc():
    if "nc" not in _NC_CACHE:
        _NC_CACHE["nc"] = build_nc()
    return _NC_CACHE["nc"]


def _make_in_maps(inputs):
    q = np.ascontiguousarray(np.asarray(inputs["query"], dtype=np.float32))
    v = np.ascontiguousarray(np.asarray(inputs["value"], dtype=np.float32))
    rp = np.ascontiguousarray(np.asarray(inputs["reference_points"], dtype=np.float32))
    aw = np.ascontiguousarray(
        np.asarray(inputs["attn_w"], dtype=np.float32).reshape(16, C, 9))
    ab = np.asarray(inputs["attn_b"], dtype=np.float32).reshape(16, 1)
    pw = np.ascontiguousarray(
        np.asarray(inputs["proj_w"], dtype=np.float32).reshape(C, C))
    pb = np.asarray(inputs["proj_b"], dtype=np.float32).reshape(C, 1)

    in_maps = []
    for b in range(8):
        rpb = rp[b].reshape(HW, NH, NP, 2)
        x0p1 = np.floor(rpb[..., 0] * W + 0.5)
        y0p1 = np.floor(rpb[..., 1] * H + 0.5)
        idx = (y0p1 * 96 + x0p1).astype(np.int16)       # [HW, NH, NP]
        gc = np.empty((HW, NH, NP, 2), np.float32)
        gc[..., 0] = x0p1
        gc[..., 1] = y0p1
        gc = np.ascontiguousarray(gc.reshape(HW, 32))
        # wrapped+replicated gather idx tensor: G[h*8+p, r, c*8+g] =
        #   idx[pix = c*128 + g*16 + (r%16), h, p]
        it = idx.reshape(NCH, 8, 16, NH, NP)             # [c, g, q, h, p]
        G = np.transpose(it, (3, 4, 2, 0, 1)).reshape(NH * NP, 1, 16, NCH * 8)
        G = np.broadcast_to(G, (NH * NP, 8, 16, NCH * 8))
        G = np.ascontiguousarray(
            G.reshape(NH * NP, 128, NCH * 8)).astype(np.int16)
        if os.environ.get("CONST_IDX") == "1":
            G = np.zeros_like(G)
        elif os.environ.get("CONST_IDX") == "2":
            # sequential: sample i gathers row i -> max HBM locality
            c = np.arange(NCH)[:, None, None]
            g = np.arange(8)[None, :, None]
            qq = np.arange(16)[None, None, :]
            seq = (c * 128 + g * 16 + qq).astype(np.int16)  # [c,g,q]
            it2 = np.broadcast_to(seq[:, :, :, None, None], (NCH, 8, 16, NH, NP))
            G2 = np.transpose(it2, (3, 4, 2, 0, 1)).reshape(NH * NP, 1, 16, NCH * 8)
            G2 = np.broadcast_to(G2, (NH * NP, 8, 16, NCH * 8))
            G = np.ascontiguousarray(G2.reshape(NH * NP, 128, NCH * 8)).astype(np.int16)
        in_maps.append({
            "query": q[b],
            "value": v[b].reshape(C, HW),
            "rp": rp[b].reshape(HW, 32),
            "attn_w": aw,
            "attn_b": ab,
            "proj_w": pw,
            "proj_b": pb,
            "gidx": G,
            "gcoord": gc,
        })
    return in_maps


def kernel(**inputs):
    nc = _get_nc()
    from concourse.bass_utils import run_bass_kernel_spmd

    in_maps = _make_in_maps(inputs)
    res = run_bass_kernel_spmd(nc, in_maps, list(range(8)))
    _NC_CACHE["exec_time_ns"] = res.exec_time_ns
    _NC_CACHE["mean_exec_time_ns"] = res.mean_exec_time_ns
    _NC_CACHE["profile_json"] = res.profile_json
    outs = [res.results[b]["out"].reshape(C, H, W) for b in range(8)]
    return np.stack(outs).astype(np.float32)


if __name__ == "__main__":
    nc = build_nc()
    n = sum(len(bb.instructions) for bb in nc.main_func.blocks)
    print("built ok, instructions:", n)
